# revision 8
# baseline (speedup 1.0000x reference)
"""Multi-head attention distributed over 8 Trainium2 NeuronCores.

Sharding: core = (batch b, head-group g) with b in 0..3, g in 0..1.
Each core computes 4 heads of one batch end-to-end (QKV projection,
scores, softmax, attn@V, output projection) and returns a partial
[2048, 512] output; the host sums the two group partials per batch.

Layout strategy (per core):
  - Host supplies X^T (d_model on partitions) so projections need no
    on-device transpose.
  - Q^T, K^T computed in [dq, tokens] layout; V in natural [tokens, dv]
    layout with an extra ones-column per head (so attn@V also produces
    the softmax denominators for free).
  - Scores are computed transposed (S^T = [k_tokens, q_tokens]) so the
    exp output P^T feeds attn@V directly with no transposes.
  - Normalization (1/sum) happens on O^T via a PE outer-product
    broadcast; biases bq/bk are per-partition ACT/DVE adds, bv is folded
    into V, bo is added via a K=1 ones matmul on the g==0 core only.
All matmuls run in float32r (full PE rate at moving dim >= 256).
"""

import numpy as np

import concourse.bacc as bacc
import concourse.mybir as mybir
import concourse.tile as tile
from concourse.bass import ds
from concourse.bass_utils import run_bass_kernel_spmd

D_MODEL, DQ, DV, H = 512, 64, 64, 8
B, M = 4, 2048
NCORES, GROUPS = 8, 2
HL = H // GROUPS            # heads per core
VW = HL * (DV + 1)          # V width incl. ones columns = 260
SCALE = float(1.0 / np.sqrt(np.float32(M)))
NKT = D_MODEL // 128        # 4 contraction tiles over d_model
NTT = M // 128              # 16 token tiles
NQC = M // 512              # 4 query chunks of 512

F32 = mybir.dt.float32
F32R = mybir.dt.float32r
AF = mybir.ActivationFunctionType
OP = mybir.AluOpType

_prog_cache = {}


def _emit_body(nc, tc, t):
    """Emit one full forward pass. `t` maps dram tensor name -> AP."""
    P = 128

    with (
        tc.tile_pool(name="consts", bufs=1) as cpool,
        tc.tile_pool(name="persist", bufs=1) as ppool,
    ):
        # ---- constants / weights ----
        wq_all = cpool.tile([P, NKT, 256], F32R, tag="wq", name="wq_all")
        wk_all = cpool.tile([P, NKT, 256], F32R, tag="wk", name="wk_all")
        wv_all = cpool.tile([P, NKT, VW], F32R, tag="wv", name="wv_all")
        wo_all = cpool.tile([64, HL, 512], F32R, tag="wo", name="wo_all")
        bqk = cpool.tile([P, 4], F32, tag="bqk", name="bqk")
        bvb = cpool.tile([P, VW], F32, tag="bvb", name="bvb")
        bo_row = cpool.tile([1, 512], F32R, tag="bo", name="bo_row")
        ones = cpool.tile([1, P], F32R, tag="ones", name="ones")

        for kt in range(NKT):
            nc.sync.dma_start(out=wq_all[:, kt, :], in_=t["wq"][ds(kt * P, P), :])
            nc.sync.dma_start(out=wk_all[:, kt, :], in_=t["wk"][ds(kt * P, P), :])
            nc.sync.dma_start(out=wv_all[:, kt, :], in_=t["wv"][ds(kt * P, P), :])
        for h in range(HL):
            nc.sync.dma_start(out=wo_all[:, h, :], in_=t["wo"][ds(h * 64, 64), :])
        nc.sync.dma_start(out=bqk[:], in_=t["bqk"][:, :])
        nc.sync.dma_start(out=bvb[:], in_=t["bvb"][:, :])
        nc.sync.dma_start(out=bo_row[:], in_=t["bo"][:, :])
        nc.sync.dma_start(out=ones[:], in_=t["ones"][:, :])

        # ---- persistent activations ----
        qT = [ppool.tile([P, M], F32R, tag=f"qT{i}", name=f"qT{i}") for i in range(2)]
        kT = [ppool.tile([P, M], F32R, tag=f"kT{i}", name=f"kT{i}") for i in range(2)]
        v_all = ppool.tile([P, NTT, VW], F32R, tag="v", name="v_all")
        o_sb = [ppool.tile([65, M], F32R, tag=f"o{h}", name=f"osb{h}") for h in range(HL)]

        # ================= Phase 1: projections =================
        with (
            tc.tile_pool(name="xt", bufs=5) as xt_pool,
            tc.tile_pool(name="psproj", bufs=1, space="PSUM") as psq,
        ):
            # Q^T and K^T : [256, 2048] as two [128, 2048] tiles each
            for which, w_all, xname, bcol, dst in (
                ("q", wq_all, "xqT", 0, qT),
                ("k", wk_all, "xkT", 2, kT),
            ):
                xts = []
                for kt in range(NKT):
                    xt_t = xt_pool.tile([P, M], F32R, tag="xt", name="xt")
                    nc.sync.dma_start(out=xt_t[:], in_=t[xname][ds(kt * P, P), :])
                    xts.append(xt_t)
                for dq in range(2):
                    pss = [psq.tile([P, 512], F32, tag=f"pq{qc}", name=f"pq{qc}") for qc in range(NQC)]
                    for kt in range(NKT):
                        for qc in range(NQC):
                            nc.tensor.matmul(
                                pss[qc][:],
                                lhsT=w_all[:, kt, ds(dq * P, P)],
                                rhs=xts[kt][:, ds(qc * 512, 512)],
                                start=(kt == 0),
                                stop=(kt == NKT - 1),
                            )
                    for qc in range(NQC):
                        nc.vector.tensor_scalar(
                            dst[dq][:, ds(qc * 512, 512)],
                            pss[qc][:],
                            bqk[:, ds(bcol + dq, 1)],
                            None,
                            OP.add,
                        )
            # V : natural layout [tokens, 260] with bias + ones columns
            xts = []
            for kt in range(NKT):
                xt_t = xt_pool.tile([P, M], F32R, tag="xt", name="xt")
                nc.sync.dma_start(out=xt_t[:], in_=t["xvT"][ds(kt * P, P), :])
                xts.append(xt_t)
            for tt in range(NTT):
                psv = psq.tile([P, VW], F32, tag="pv", name="pv")
                for kt in range(NKT):
                    nc.tensor.matmul(
                        psv[:],
                        lhsT=xts[kt][:, ds(tt * P, P)],
                        rhs=wv_all[:, kt, :],
                        start=(kt == 0),
                        stop=(kt == NKT - 1),
                    )
                nc.vector.tensor_tensor(v_all[:, tt, :], psv[:], bvb[:], OP.add)

        # ================= Phase 2: attention =================
        with (
            tc.tile_pool(name="pT", bufs=2) as pt_pool,
            tc.tile_pool(name="psatt", bufs=1, space="PSUM") as psa,
        ):
            for hp in range(2):          # head pair (dq tile)
                for qcp in range(2):     # pair of 512-wide q chunks
                    ops = {
                        (h01, qci): psa.tile([65, 512], F32, tag=f"po{h01}{qci}", name=f"po{h01}{qci}")
                        for h01 in range(2)
                        for qci in range(2)
                    }
                    for j in range(NTT):
                        for h01 in range(2):
                            h = hp * 2 + h01
                            sps = psa.tile([P, 1024], F32, tag=f"ps{h01}", name=f"ps{h01}")
                            for qci in range(2):
                                qc = qcp * 2 + qci
                                nc.tensor.matmul(
                                    sps[:, ds(qci * 512, 512)],
                                    lhsT=kT[hp][ds(h01 * 64, 64), ds(j * P, P)],
                                    rhs=qT[hp][ds(h01 * 64, 64), ds(qc * 512, 512)],
                                    start=True,
                                    stop=True,
                                )
                            pt = pt_pool.tile([P, 1024], F32R, tag=f"pt{h01}", name=f"pt{h01}")
                            nc.scalar.activation(pt[:], sps[:], AF.Exp, scale=SCALE)
                            for qci in range(2):
                                nc.tensor.matmul(
                                    ops[(h01, qci)][:],
                                    lhsT=v_all[:, j, ds(h * 65, 65)],
                                    rhs=pt[:, ds(qci * 512, 512)],
                                    start=(j == 0),
                                    stop=(j == NTT - 1),
                                )
                    for h01 in range(2):
                        for qci in range(2):
                            h = hp * 2 + h01
                            qc = qcp * 2 + qci
                            nc.vector.tensor_copy(
                                o_sb[h][:, ds(qc * 512, 512)], ops[(h01, qci)][:]
                            )

        # ================= Phase 3: normalize + output proj =================
        with (
            tc.tile_pool(name="fin", bufs=1) as fpool,
            tc.tile_pool(name="outb", bufs=3) as opool,
            tc.tile_pool(name="psfin", bufs=2, space="PSUM") as psf,
        ):
            sr = fpool.tile([P, P], F32R, tag="sr", name="sr")
            r_flat = fpool.tile([1, HL * M], F32R, tag="rflat", name="r_flat")
            # gather softmax sums (row 64 of each head's O^T) into [128, 64]
            for h in range(HL):
                nc.sync.dma_start(out=sr[:, ds(h * 16, 16)], in_=o_sb[h][ds(64, 1), :])
            with nc.allow_low_precision(reason="f32r is bit-identical to f32"):
                nc.vector.reciprocal(sr[:, ds(64, 64)], sr[:, ds(0, 64)])
            for h in range(HL):
                nc.sync.dma_start(
                    out=r_flat[ds(0, 1), ds(h * M, M)], in_=sr[:, ds(64 + h * 16, 16)]
                )
            # normalize O^T in place: O^T *= broadcast(r)
            for h in range(HL):
                for qc in range(NQC):
                    rb = psf.tile([64, 512], F32, tag="rb", name="rb")
                    nc.tensor.matmul(
                        rb[:],
                        lhsT=ones[ds(0, 1), ds(0, 64)],
                        rhs=r_flat[ds(0, 1), ds(h * M + qc * 512, 512)],
                        start=True,
                        stop=True,
                    )
                    nc.vector.tensor_tensor(
                        o_sb[h][ds(0, 64), ds(qc * 512, 512)],
                        o_sb[h][ds(0, 64), ds(qc * 512, 512)],
                        rb[:],
                        OP.mult,
                    )
            # out[tok, :] = sum_h O_h @ Wo_h + bo
            for tt in range(NTT):
                fp = psf.tile([P, 512], F32, tag="f", name="f")
                for h in range(HL):
                    nc.tensor.matmul(
                        fp[:],
                        lhsT=o_sb[h][ds(0, 64), ds(tt * P, P)],
                        rhs=wo_all[:, h, :],
                        start=(h == 0),
                        stop=False,
                    )
                nc.tensor.matmul(
                    fp[:],
                    lhsT=ones[ds(0, 1), :],
                    rhs=bo_row[ds(0, 1), :],
                    start=False,
                    stop=True,
                )
                ob = opool.tile([P, 512], F32, tag="ob", name="ob")
                nc.vector.tensor_copy(ob[:], fp[:])
                nc.sync.dma_start(out=t["out"][ds(tt * P, P), :], in_=ob[:])


def _build(reps=1):
    if reps in _prog_cache:
        return _prog_cache[reps]
    nc = bacc.Bacc(
        "TRN2",
        target_bir_lowering=False,
        debug=False,
        enable_asserts=False,
        num_devices=NCORES,
    )
    t = {}
    for name, shape, dt in (
        ("xqT", (D_MODEL, M), F32R),
        ("xkT", (D_MODEL, M), F32R),
        ("xvT", (D_MODEL, M), F32R),
        ("wq", (D_MODEL, 256), F32R),
        ("wk", (D_MODEL, 256), F32R),
        ("wv", (D_MODEL, VW), F32R),
        ("wo", (256, 512), F32R),
        ("bqk", (128, 4), F32),
        ("bvb", (128, VW), F32),
        ("bo", (1, 512), F32R),
        ("ones", (1, 128), F32R),
    ):
        t[name] = nc.dram_tensor(name, shape, dt, kind="ExternalInput").ap()
    t["out"] = nc.dram_tensor("out", (M, D_MODEL), F32, kind="ExternalOutput").ap()

    with tile.TileContext(nc) as tc:
        for _ in range(reps):
            _emit_body(nc, tc, t)
    nc.compile()
    _prog_cache[reps] = (nc, t)
    return _prog_cache[reps]


def shard_inputs(query, key, value, Wq, bq, Wk, bk, Wv, bv, Wo, bo):
    query, key, value, Wq, bq, Wk, bk, Wv, bv, Wo, bo = (
        np.asarray(a, dtype=np.float32)
        for a in (query, key, value, Wq, bq, Wk, bk, Wv, bv, Wo, bo)
    )
    in_maps = []
    for b in range(B):
        xqT = np.ascontiguousarray(query[b].T)
        xkT = np.ascontiguousarray(key[b].T)
        xvT = np.ascontiguousarray(value[b].T)
        for g in range(GROUPS):
            hs = slice(g * 256, (g + 1) * 256)
            wv_ext = np.zeros((D_MODEL, VW), np.float32)
            bv_ext = np.zeros((VW,), np.float32)
            for i in range(HL):
                gh = g * HL + i
                wv_ext[:, i * 65 : i * 65 + 64] = Wv[:, gh * 64 : (gh + 1) * 64]
                bv_ext[i * 65 : i * 65 + 64] = bv[gh * 64 : (gh + 1) * 64]
                bv_ext[i * 65 + 64] = 1.0
            bqk = np.concatenate(
                [bq[hs].reshape(2, 128).T, bk[hs].reshape(2, 128).T], axis=1
            )
            in_maps.append(
                {
                    "xqT": xqT,
                    "xkT": xkT,
                    "xvT": xvT,
                    "wq": np.ascontiguousarray(Wq[:, hs]),
                    "wk": np.ascontiguousarray(Wk[:, hs]),
                    "wv": wv_ext,
                    "wo": np.ascontiguousarray(Wo[hs, :]),
                    "bqk": np.ascontiguousarray(bqk),
                    "bvb": np.ascontiguousarray(
                        np.broadcast_to(bv_ext, (128, VW))
                    ).copy(),
                    "bo": (bo if g == 0 else np.zeros_like(bo)).reshape(1, 512).copy(),
                    "ones": np.ones((1, 128), np.float32),
                }
            )
    return in_maps


def unshard_outputs(results):
    return np.stack(
        [results[2 * b]["out"] + results[2 * b + 1]["out"] for b in range(B)]
    )


def kernel(query, key, value, Wq, bq, Wk, bk, Wv, bv, Wo, bo):
    nc, _ = _build(reps=1)
    in_maps = shard_inputs(query, key, value, Wq, bq, Wk, bk, Wv, bv, Wo, bo)
    res = run_bass_kernel_spmd(nc, in_maps, core_ids=list(range(NCORES)))
    return unshard_outputs(res.results)


# revision 11
# speedup vs baseline: 581.1344x; 581.1344x over previous
"""Multi-head attention distributed over 8 Trainium2 NeuronCores.

Sharding: core = (batch b, head-group g) with b in 0..3, g in 0..1.
Each core computes 4 heads of one batch end-to-end (QKV projection,
scores, softmax, attn@V, output projection) and returns a partial
[2048, 512] output; the host sums the two group partials per batch.

Layout strategy (per core):
  - Host supplies X^T (d_model on partitions) so projections need no
    on-device transpose; X^T streams through SBUF in [128, 1024] chunks.
  - Q^T, K^T computed in [dq, tokens] layout; V in natural [tokens, dv]
    layout with an extra ones-column per head (so attn@V also produces
    the softmax denominators for free).
  - Scores are computed transposed (S^T = [k_tokens, q_tokens]) so the
    exp output P^T feeds attn@V directly with no transposes. The two
    heads of a dq-tile issue adjacent 64x128 row-tiled matmuls (PE array
    tiles T0/T8 run concurrently) and share one [128, 1024] exp op.
  - Normalization (1/sum) happens on O^T via a PE outer-product
    broadcast; biases bq/bk are per-partition DVE adds, bv is folded
    into V, bo is added via a K=1 ones matmul on the g==0 core only.
All matmuls run in float32r (full PE rate at moving dim >= 256).

Schedule: head-pair-0 K/Q projections sweep kt-outer over all 8 PSUM
banks right behind the streaming input DMAs, so attention starts as
early as possible. V projection and the head-pair-1 K/Q projections run
k-inner on the two out-proj/norm banks, overlapping early attention.
Attention runs one (head-pair, q-chunk) unit at a time; normalization
follows each unit; output projection is split (heads 0/1 + bo store,
heads 2/3 accumulate into DRAM via CCE add-DMA) and overlaps attention.
"""

import numpy as np

import concourse.bacc as bacc
import concourse.mybir as mybir
import concourse.tile as tile
from concourse.bass import ds
from concourse.bass_utils import run_bass_kernel_spmd

D_MODEL, DQ, DV, H = 512, 64, 64, 8
B, M = 4, 2048
NCORES, GROUPS = 8, 2
HL = H // GROUPS            # heads per core
VW = HL * (DV + 1)          # V width incl. ones columns = 260
SCALE = float(1.0 / np.sqrt(np.float32(M)))
NKT = D_MODEL // 128        # 4 contraction tiles over d_model
NTT = M // 128              # 16 token tiles
NQC = M // 512              # 4 query chunks of 512

F32 = mybir.dt.float32
F32R = mybir.dt.float32r
AF = mybir.ActivationFunctionType
OP = mybir.AluOpType

_prog_cache = {}


def _emit_body(nc, tc, t):
    """Emit one full forward pass. `t` maps dram tensor name -> AP."""
    P = 128

    with (
        tc.tile_pool(name="consts", bufs=1) as cpool,
        tc.tile_pool(name="persist", bufs=1) as ppool,
    ):
        # ---- constants / weights (small DMAs first) ----
        wq_all = cpool.tile([P, NKT, 256], F32R, tag="wq", name="wq_all")
        wk_all = cpool.tile([P, NKT, 256], F32R, tag="wk", name="wk_all")
        wv_all = cpool.tile([P, NKT, VW], F32R, tag="wv", name="wv_all")
        wo_all = cpool.tile([64, HL, 512], F32R, tag="wo", name="wo_all")
        bmisc = cpool.tile([P, 264], F32, tag="bmisc", name="bmisc")
        misc = cpool.tile([1, 640], F32R, tag="misc", name="misc")
        bqk = bmisc[:, ds(0, 4)]
        bvb = bmisc[:, ds(4, VW)]
        bo_row = misc[ds(0, 1), ds(0, 512)]
        ones = misc[ds(0, 1), ds(512, P)]

        for kt in range(NKT):
            nc.sync.dma_start(out=wk_all[:, kt, :], in_=t["wk"][ds(kt * P, P), :])
            nc.sync.dma_start(out=wq_all[:, kt, :], in_=t["wq"][ds(kt * P, P), :])
            nc.sync.dma_start(out=wv_all[:, kt, :], in_=t["wv"][ds(kt * P, P), :])
        for h in range(HL):
            nc.sync.dma_start(out=wo_all[:, h, :], in_=t["wo"][ds(h * 64, 64), :])
        nc.sync.dma_start(out=bmisc[:], in_=t["bmisc"][:, :])
        nc.sync.dma_start(out=misc[:], in_=t["misc"][:, :])

        # ---- persistent activations ----
        qT = [ppool.tile([P, M], F32R, tag=f"qT{i}", name=f"qT{i}") for i in range(2)]
        kT = [ppool.tile([P, M], F32R, tag=f"kT{i}", name=f"kT{i}") for i in range(2)]
        v_all = ppool.tile([P, NTT, VW], F32R, tag="v", name="v_all")
        o_sb = [ppool.tile([65, M], F32R, tag=f"o{h}", name=f"osb{h}") for h in range(HL)]

        with tc.tile_pool(name="xc", bufs=8) as xc_pool:
            # ---- dq-tile-0 K and Q projections: kt-outer over all 8 banks ----
            with tc.tile_pool(name="psq8", bufs=1, space="PSUM") as psq8:
                pss = {
                    (w, cg): psq8.tile(
                        [P, 1024], F32, tag=f"p{w}{cg}", name=f"p{w}{cg}"
                    )
                    for w in ("k", "q")
                    for cg in range(2)
                }
                for kt in range(NKT):
                    for w, w_all, xname in (
                        ("k", wk_all, "xkT"),
                        ("q", wq_all, "xqT"),
                    ):
                        xch = []
                        for cg in range(2):
                            c = xc_pool.tile([P, 1024], F32R, tag="xc", name="xc")
                            nc.sync.dma_start(
                                out=c[:],
                                in_=t[xname][ds(kt * P, P), ds(cg * 1024, 1024)],
                            )
                            xch.append(c)
                        for cg in range(2):
                            for qh in range(2):
                                nc.tensor.matmul(
                                    pss[(w, cg)][:, ds(qh * 512, 512)],
                                    lhsT=w_all[:, kt, ds(0, P)],
                                    rhs=xch[cg][:, ds(qh * 512, 512)],
                                    start=(kt == 0),
                                    stop=(kt == NKT - 1),
                                )
                for w, dst, bcol in (("k", kT, 2), ("q", qT, 0)):
                    for cg in range(2):
                        nc.vector.tensor_scalar(
                            dst[0][:, ds(cg * 1024, 1024)],
                            pss[(w, cg)][:],
                            bqk[:, ds(bcol, 1)],
                            None,
                            OP.add,
                        )

            # ---- attention; V and dq-tile-1 projections borrow psf banks ----
            with (
                tc.tile_pool(name="pT", bufs=4) as pt_pool,
                tc.tile_pool(name="fin", bufs=1) as fpool,
                tc.tile_pool(name="outb", bufs=3) as opool,
                tc.tile_pool(name="psatt", bufs=1, space="PSUM") as psa,
                tc.tile_pool(name="psfin", bufs=1, space="PSUM") as psf,
            ):
                sr = fpool.tile([P, P], F32R, tag="sr", name="sr")
                # preload the exp table set while DMAs stream
                nc.scalar.activation(sr[ds(0, 1), ds(0, 1)], misc[ds(0, 1), ds(0, 1)], AF.Exp)

                # V projection: k-inner on the psf "f" bank
                for cg in range(2):
                    xvch = []
                    for kt in range(NKT):
                        c = xc_pool.tile([P, 1024], F32R, tag="xc", name="xvc")
                        nc.sync.dma_start(
                            out=c[:],
                            in_=t["xvT"][ds(kt * P, P), ds(cg * 1024, 1024)],
                        )
                        xvch.append(c)
                    for tl in range(8):
                        tt = cg * 8 + tl
                        psv = psf.tile([P, 512], F32, tag="f", name="pv")
                        for kt in range(NKT):
                            nc.tensor.matmul(
                                psv[:, ds(0, VW)],
                                lhsT=xvch[kt][:, ds(tl * P, P)],
                                rhs=wv_all[:, kt, :],
                                start=(kt == 0),
                                stop=(kt == NKT - 1),
                            )
                        nc.vector.tensor_tensor(
                            v_all[:, tt, :], psv[:, ds(0, VW)], bvb[:], OP.add
                        )

                # dq-tile-1 K and Q projections: k-inner on the psf "rb" bank
                for w, w_all, xname, dst, bcol in (
                    ("k", wk_all, "xkT", kT, 2),
                    ("q", wq_all, "xqT", qT, 0),
                ):
                    for cg in range(2):
                        xch = []
                        for kt in range(NKT):
                            c = xc_pool.tile([P, 1024], F32R, tag="xc", name="xr")
                            nc.sync.dma_start(
                                out=c[:],
                                in_=t[xname][ds(kt * P, P), ds(cg * 1024, 1024)],
                            )
                            xch.append(c)
                        for qh in range(2):
                            ps = psf.tile([P, 512], F32, tag="rb", name="pp")
                            for kt in range(NKT):
                                nc.tensor.matmul(
                                    ps[:],
                                    lhsT=w_all[:, kt, ds(P, P)],
                                    rhs=xch[kt][:, ds(qh * 512, 512)],
                                    start=(kt == 0),
                                    stop=(kt == NKT - 1),
                                )
                            nc.vector.tensor_scalar(
                                dst[1][:, ds((cg * 2 + qh) * 512, 512)],
                                ps[:],
                                bqk[:, ds(bcol + 1, 1)],
                                None,
                                OP.add,
                            )

                def attn_unit(hp, qc):
                    po = [
                        psa.tile([65, 512], F32, tag=f"po{h01}", name=f"po{h01}")
                        for h01 in range(2)
                    ]
                    for jp in range(NTT // 2):
                        sp2, pt2 = [], []
                        for jj in range(2):
                            j = 2 * jp + jj
                            sps = psa.tile([P, 1024], F32, tag="ps", name="ps", bufs=2)
                            for h01 in range(2):
                                nc.tensor.matmul(
                                    sps[:, ds(h01 * 512, 512)],
                                    lhsT=kT[hp][ds(h01 * 64, 64), ds(j * P, P)],
                                    rhs=qT[hp][ds(h01 * 64, 64), ds(qc * 512, 512)],
                                    start=True,
                                    stop=True,
                                )
                            sp2.append(sps)
                        for jj in range(2):
                            pt = pt_pool.tile([P, 1024], F32R, tag="pt", name="pt")
                            nc.scalar.activation(pt[:], sp2[jj][:], AF.Exp, scale=SCALE)
                            pt2.append(pt)
                        for jj in range(2):
                            j = 2 * jp + jj
                            for h01 in range(2):
                                h = hp * 2 + h01
                                nc.tensor.matmul(
                                    po[h01][:],
                                    lhsT=v_all[:, j, ds(h * 65, 65)],
                                    rhs=pt2[jj][:, ds(h01 * 512, 512)],
                                    start=(j == 0),
                                    stop=(j == NTT - 1),
                                )
                    for h01 in range(2):
                        h = hp * 2 + h01
                        nc.vector.tensor_copy(o_sb[h][:, ds(qc * 512, 512)], po[h01][:])

                def norm_unit(hp, qc):
                    for h01 in range(2):
                        h = hp * 2 + h01
                        idx = h * 4 + qc
                        nc.sync.dma_start(
                            out=sr[:, ds(idx * 4, 4)],
                            in_=o_sb[h][ds(64, 1), ds(qc * 512, 512)],
                        )
                        with nc.allow_low_precision(reason="f32r == f32 bits"):
                            nc.vector.reciprocal(
                                sr[:, ds(64 + idx * 4, 4)], sr[:, ds(idx * 4, 4)]
                            )
                        rr = fpool.tile([1, 512], F32R, tag="rrow", name="rrow", bufs=2)
                        nc.sync.dma_start(
                            out=rr[ds(0, 1), :], in_=sr[:, ds(64 + idx * 4, 4)]
                        )
                        rb = psf.tile([64, 512], F32, tag="rb", name="rb")
                        nc.tensor.matmul(
                            rb[:],
                            lhsT=ones[ds(0, 1), ds(0, 64)],
                            rhs=rr[ds(0, 1), :],
                            start=True,
                            stop=True,
                        )
                        nc.vector.tensor_tensor(
                            o_sb[h][ds(0, 64), ds(qc * 512, 512)],
                            o_sb[h][ds(0, 64), ds(qc * 512, 512)],
                            rb[:],
                            OP.mult,
                        )

                def outproj_a(tts):
                    # heads 0,1 + bo -> store to DRAM out
                    for tt in tts:
                        fp = psf.tile([P, 512], F32, tag="f", name="f")
                        for h in (0, 1):
                            nc.tensor.matmul(
                                fp[:],
                                lhsT=o_sb[h][ds(0, 64), ds(tt * P, P)],
                                rhs=wo_all[:, h, :],
                                start=(h == 0),
                                stop=False,
                            )
                        nc.tensor.matmul(
                            fp[:], lhsT=ones, rhs=bo_row, start=False, stop=True
                        )
                        ob = opool.tile([P, 512], F32, tag="ob", name="ob")
                        nc.vector.tensor_copy(ob[:], fp[:])
                        nc.sync.dma_start(out=t["out"][ds(tt * P, P), :], in_=ob[:])

                def outproj_b(tts):
                    # heads 2,3 accumulate into DRAM via CCE add
                    for tt in tts:
                        fp = psf.tile([P, 512], F32, tag="f", name="f")
                        for h in (2, 3):
                            nc.tensor.matmul(
                                fp[:],
                                lhsT=o_sb[h][ds(0, 64), ds(tt * P, P)],
                                rhs=wo_all[:, h, :],
                                start=(h == 2),
                                stop=(h == 3),
                            )
                        ob = opool.tile([P, 512], F32, tag="ob", name="ob")
                        nc.vector.tensor_copy(ob[:], fp[:])
                        nc.gpsimd.dma_start(
                            out=t["out"][ds(tt * P, P), :],
                            in_=ob[:],
                            accum_op=OP.add,
                        )

                for hp in range(2):
                    for qc in range(NQC):
                        attn_unit(hp, qc)
                        norm_unit(hp, qc)
                        if hp == 1 and qc == 1:
                            outproj_b(range(8))
                    if hp == 0:
                        outproj_a(range(NTT))
                outproj_b(range(8, NTT))


def _build(reps=1):
    if reps in _prog_cache:
        return _prog_cache[reps]
    nc = bacc.Bacc(
        "TRN2",
        target_bir_lowering=False,
        debug=False,
        enable_asserts=False,
        num_devices=NCORES,
    )
    t = {}
    for name, shape, dt in (
        ("xqT", (D_MODEL, M), F32R),
        ("xkT", (D_MODEL, M), F32R),
        ("xvT", (D_MODEL, M), F32R),
        ("wq", (D_MODEL, 256), F32R),
        ("wk", (D_MODEL, 256), F32R),
        ("wv", (D_MODEL, VW), F32R),
        ("wo", (256, 512), F32R),
        ("bmisc", (128, 264), F32),
        ("misc", (1, 640), F32R),
    ):
        t[name] = nc.dram_tensor(name, shape, dt, kind="ExternalInput").ap()
    t["out"] = nc.dram_tensor("out", (M, D_MODEL), F32, kind="ExternalOutput").ap()

    with tile.TileContext(nc) as tc:
        for _ in range(reps):
            _emit_body(nc, tc, t)
    nc.compile()
    _prog_cache[reps] = (nc, t)
    return _prog_cache[reps]


def shard_inputs(query, key, value, Wq, bq, Wk, bk, Wv, bv, Wo, bo):
    query, key, value, Wq, bq, Wk, bk, Wv, bv, Wo, bo = (
        np.asarray(a, dtype=np.float32)
        for a in (query, key, value, Wq, bq, Wk, bk, Wv, bv, Wo, bo)
    )
    in_maps = []
    for b in range(B):
        xqT = np.ascontiguousarray(query[b].T)
        xkT = np.ascontiguousarray(key[b].T)
        xvT = np.ascontiguousarray(value[b].T)
        for g in range(GROUPS):
            hs = slice(g * 256, (g + 1) * 256)
            wv_ext = np.zeros((D_MODEL, VW), np.float32)
            bv_ext = np.zeros((VW,), np.float32)
            for i in range(HL):
                gh = g * HL + i
                wv_ext[:, i * 65 : i * 65 + 64] = Wv[:, gh * 64 : (gh + 1) * 64]
                bv_ext[i * 65 : i * 65 + 64] = bv[gh * 64 : (gh + 1) * 64]
                bv_ext[i * 65 + 64] = 1.0
            bmisc = np.zeros((128, 264), np.float32)
            bmisc[:, 0:2] = bq[hs].reshape(2, 128).T
            bmisc[:, 2:4] = bk[hs].reshape(2, 128).T
            bmisc[:, 4:] = bv_ext
            misc = np.zeros((1, 640), np.float32)
            if g == 0:
                misc[0, 0:512] = bo
            misc[0, 512:640] = 1.0
            in_maps.append(
                {
                    "xqT": xqT,
                    "xkT": xkT,
                    "xvT": xvT,
                    "wq": np.ascontiguousarray(Wq[:, hs]),
                    "wk": np.ascontiguousarray(Wk[:, hs]),
                    "wv": wv_ext,
                    "wo": np.ascontiguousarray(Wo[hs, :]),
                    "bmisc": bmisc,
                    "misc": misc,
                }
            )
    return in_maps


def unshard_outputs(results):
    return np.stack(
        [results[2 * b]["out"] + results[2 * b + 1]["out"] for b in range(B)]
    )


def kernel(query, key, value, Wq, bq, Wk, bk, Wv, bv, Wo, bo):
    nc, _ = _build(reps=1)
    in_maps = shard_inputs(query, key, value, Wq, bq, Wk, bk, Wv, bv, Wo, bo)
    res = run_bass_kernel_spmd(nc, in_maps, core_ids=list(range(NCORES)))
    return unshard_outputs(res.results)


# revision 12
# speedup vs baseline: 949.5133x; 1.6339x over previous
"""Multi-head attention distributed over 8 Trainium2 NeuronCores.

Sharding: core = (batch b, head-group g); each core computes 4 heads of
one batch end-to-end and returns a partial [2048, 512] output; the host
sums the two group partials per batch and adds the constant epilogue
vector bv @ Wo + bo (exact, since softmax rows sum to 1).

v4: all matmul operands are bf16 (host ships bf16 X^T and weights;
separate LDWEIGHTS amortizes/hides, input DMA halves). K^T is stored
per-head zero-padded ([128, 2048] with the other head's partition half
zeroed) so every matmul runs K=128 — no PE tiling-mode switches and no
partition shifts. PSUM accumulation and the softmax-sum/reciprocal path
stay f32/f32r for accuracy. Scores are computed transposed so the exp
output P^T feeds attn@V directly; a ones-column in V yields softmax
sums for free; 1/sum is applied via a PE outer-product broadcast.

Schedule: head-pair-0 K/Q projections sweep kt-outer over all 8 PSUM
banks behind the streaming input DMAs; V and head-pair-1 projections
run k-inner on the 2 out-proj/norm banks overlapping early attention.
Attention runs one (head, q-pair) unit at a time; normalization follows
each unit; output projection is split (heads 0/1 store, heads 2/3
CCE-accumulate into DRAM) and overlaps later attention.
"""

import numpy as np
import ml_dtypes

import concourse.bacc as bacc
import concourse.mybir as mybir
import concourse.tile as tile
from concourse.bass import ds
from concourse.bass_utils import run_bass_kernel_spmd

D_MODEL, DQ, DV, H = 512, 64, 64, 8
B, M = 4, 2048
NCORES, GROUPS = 8, 2
HL = H // GROUPS            # heads per core
VW = HL * (DV + 1)          # V width incl. ones columns = 260
SCALE = float(1.0 / np.sqrt(np.float32(M)))
NKT = D_MODEL // 128        # 4 contraction tiles over d_model
NTT = M // 128              # 16 token tiles
NQC = M // 512              # 4 query chunks of 512

F32 = mybir.dt.float32
F32R = mybir.dt.float32r
BF16 = mybir.dt.bfloat16
AF = mybir.ActivationFunctionType
OP = mybir.AluOpType

_prog_cache = {}


def _emit_body(nc, tc, t):
    P = 128

    with (
        tc.tile_pool(name="consts", bufs=1) as cpool,
        tc.tile_pool(name="persist", bufs=1) as ppool,
    ):
        wq_all = cpool.tile([P, NKT, 256], BF16, tag="wq", name="wq_all")
        wk_all = cpool.tile([P, NKT, 256], BF16, tag="wk", name="wk_all")
        wv_all = cpool.tile([P, NKT, VW], BF16, tag="wv", name="wv_all")
        wo_all = cpool.tile([P, HL, 512], BF16, tag="wo", name="wo_all")
        bmisc = cpool.tile([P, 264], F32, tag="bmisc", name="bmisc")
        misc = cpool.tile([1, 640], F32R, tag="misc", name="misc")
        bqk = bmisc[:, ds(0, 4)]
        onespat = bmisc[:, ds(4, VW)]
        ones = misc[ds(0, 1), ds(512, P)]

        for kt in range(NKT):
            nc.sync.dma_start(out=wk_all[:, kt, :], in_=t["wk"][ds(kt * P, P), :])
            nc.sync.dma_start(out=wq_all[:, kt, :], in_=t["wq"][ds(kt * P, P), :])
            nc.sync.dma_start(out=wv_all[:, kt, :], in_=t["wv"][ds(kt * P, P), :])
        for h in range(HL):
            nc.sync.dma_start(out=wo_all[ds(0, 64), h, :], in_=t["wo"][ds(h * 64, 64), :])
        nc.sync.dma_start(out=bmisc[:], in_=t["bmisc"][:, :])
        nc.sync.dma_start(out=misc[:], in_=t["misc"][:, :])
        nc.vector.memset(wo_all[ds(64, 64), :, :], 0.0)

        # persistent activations (kTh per-head zero-padded; o_sb zero-padded)
        qT = [ppool.tile([P, M], BF16, tag=f"qT{i}", name=f"qT{i}") for i in range(2)]
        kTh = [ppool.tile([P, M], BF16, tag=f"kTh{i}", name=f"kTh{i}") for i in range(HL)]
        v_all = ppool.tile([P, NTT, VW], BF16, tag="v", name="v_all")
        o_sb = [ppool.tile([P, M], BF16, tag=f"o{h}", name=f"osb{h}") for h in range(HL)]
        for h in range(HL):
            z0, z1 = (64, 64) if h % 2 == 0 else (0, 64)
            nc.vector.memset(kTh[h][ds(z0, z1), :], 0.0)
            nc.vector.memset(o_sb[h][ds(64, 64), :], 0.0)

        with tc.tile_pool(name="xc", bufs=8) as xc_pool:
            xch = {}
            # ---- head-pair-0 K and Q projections: kt-outer over 8 banks ----
            with tc.tile_pool(name="psq8", bufs=1, space="PSUM") as psq8:
                pss = {
                    (w, cg): psq8.tile([P, 1024], F32, tag=f"p{w}{cg}", name=f"p{w}{cg}")
                    for w in ("k", "q")
                    for cg in range(2)
                }
                for kt in range(NKT):
                    for w, w_all, xname in (("k", wk_all, "xkT"), ("q", wq_all, "xqT")):
                        c = xc_pool.tile([P, M], BF16, tag="xc", name="xc")
                        nc.sync.dma_start(out=c[:], in_=t[xname][ds(kt * P, P), :])
                        xch[(w, kt)] = c
                        for qc in range(NQC):
                            nc.tensor.matmul(
                                pss[(w, qc // 2)][:, ds((qc % 2) * 512, 512)],
                                lhsT=w_all[:, kt, ds(0, P)],
                                rhs=c[:, ds(qc * 512, 512)],
                                start=(kt == 0),
                                stop=(kt == NKT - 1),
                            )
                for cg in range(2):
                    nc.vector.tensor_scalar(
                        qT[0][:, ds(cg * 1024, 1024)],
                        pss[("q", cg)][:], bqk[:, ds(0, 1)], None, OP.add,
                    )
                    nc.vector.tensor_scalar(
                        kTh[0][ds(0, 64), ds(cg * 1024, 1024)],
                        pss[("k", cg)][ds(0, 64), :], bqk[ds(0, 64), ds(2, 1)], None, OP.add,
                    )
                    nc.vector.tensor_scalar(
                        kTh[1][ds(64, 64), ds(cg * 1024, 1024)],
                        pss[("k", cg)][ds(64, 64), :], bqk[ds(64, 64), ds(2, 1)], None, OP.add,
                    )

            with (
                tc.tile_pool(name="pT", bufs=4) as pt_pool,
                tc.tile_pool(name="fin", bufs=1) as fpool,
                tc.tile_pool(name="outb", bufs=3) as opool,
                tc.tile_pool(name="psatt", bufs=1, space="PSUM") as psa,
                tc.tile_pool(name="psfin", bufs=1, space="PSUM") as psf,
            ):
                sr = fpool.tile([P, P], F32R, tag="sr", name="sr")
                nc.scalar.activation(sr[ds(0, 1), ds(0, 1)], misc[ds(0, 1), ds(0, 1)], AF.Exp)

                # V projection: k-inner on the psf "f" bank
                xv = []
                for kt in range(NKT):
                    c = xc_pool.tile([P, M], BF16, tag="xv", name="xv", bufs=4)
                    nc.sync.dma_start(out=c[:], in_=t["xvT"][ds(kt * P, P), :])
                    xv.append(c)
                for tt in range(NTT):
                    psv = psf.tile([P, 512], F32, tag="f", name="pv")
                    for kt in range(NKT):
                        nc.tensor.matmul(
                            psv[:, ds(0, VW)],
                            lhsT=xv[kt][:, ds(tt * P, P)],
                            rhs=wv_all[:, kt, :],
                            start=(kt == 0),
                            stop=(kt == NKT - 1),
                        )
                    nc.vector.tensor_tensor(
                        v_all[:, tt, :], psv[:, ds(0, VW)], onespat[:], OP.add
                    )

                # head-pair-1 K/Q projections: k-inner on the psf "rb" bank
                for w, w_all, bcol in (("k", wk_all, 2), ("q", wq_all, 0)):
                    for qc in range(NQC):
                        ps = psf.tile([P, 512], F32, tag="rb", name="pp")
                        for kt in range(NKT):
                            nc.tensor.matmul(
                                ps[:],
                                lhsT=w_all[:, kt, ds(P, P)],
                                rhs=xch[(w, kt)][:, ds(qc * 512, 512)],
                                start=(kt == 0),
                                stop=(kt == NKT - 1),
                            )
                        if w == "q":
                            nc.vector.tensor_scalar(
                                qT[1][:, ds(qc * 512, 512)],
                                ps[:], bqk[:, ds(1, 1)], None, OP.add,
                            )
                        else:
                            nc.vector.tensor_scalar(
                                kTh[2][ds(0, 64), ds(qc * 512, 512)],
                                ps[ds(0, 64), :], bqk[ds(0, 64), ds(3, 1)], None, OP.add,
                            )
                            nc.vector.tensor_scalar(
                                kTh[3][ds(64, 64), ds(qc * 512, 512)],
                                ps[ds(64, 64), :], bqk[ds(64, 64), ds(3, 1)], None, OP.add,
                            )

                def attn_unit(h, qcp):
                    hp = h // 2
                    po = [
                        psa.tile([65, 512], F32, tag=f"po{qci}", name=f"po{qci}")
                        for qci in range(2)
                    ]
                    for j in range(NTT):
                        sps = psa.tile([P, 1024], F32, tag="ps", name="ps", bufs=2)
                        for qci in range(2):
                            qc = qcp * 2 + qci
                            nc.tensor.matmul(
                                sps[:, ds(qci * 512, 512)],
                                lhsT=kTh[h][:, ds(j * P, P)],
                                rhs=qT[hp][:, ds(qc * 512, 512)],
                                start=True,
                                stop=True,
                            )
                        pt = pt_pool.tile([P, 1024], BF16, tag="pt", name="pt")
                        nc.scalar.activation(pt[:], sps[:], AF.Exp, scale=SCALE)
                        for qci in range(2):
                            nc.tensor.matmul(
                                po[qci][:],
                                lhsT=v_all[:, j, ds(h * 65, 65)],
                                rhs=pt[:, ds(qci * 512, 512)],
                                start=(j == 0),
                                stop=(j == NTT - 1),
                            )
                    for qci in range(2):
                        qc = qcp * 2 + qci
                        nc.vector.tensor_copy(
                            o_sb[h][ds(0, 64), ds(qc * 512, 512)], po[qci][ds(0, 64), :]
                        )
                        srow = fpool.tile([1, 512], F32R, tag="srow", name="srow", bufs=2)
                        nc.vector.tensor_copy(srow[:], po[qci][ds(64, 1), :])
                        idx = h * 4 + qc
                        nc.sync.dma_start(out=sr[:, ds(idx * 4, 4)], in_=srow[:])
                        with nc.allow_low_precision(reason="f32r == f32 bits"):
                            nc.vector.reciprocal(
                                sr[:, ds(64 + idx * 4, 4)], sr[:, ds(idx * 4, 4)]
                            )
                        rr = fpool.tile([1, 512], F32R, tag="rrow", name="rrow", bufs=2)
                        nc.sync.dma_start(out=rr[:], in_=sr[:, ds(64 + idx * 4, 4)])
                        rb = psf.tile([64, 512], F32, tag="rb", name="rb")
                        nc.tensor.matmul(
                            rb[:],
                            lhsT=ones[ds(0, 1), ds(0, 64)],
                            rhs=rr[ds(0, 1), :],
                            start=True,
                            stop=True,
                        )
                        nc.vector.tensor_tensor(
                            o_sb[h][ds(0, 64), ds(qc * 512, 512)],
                            o_sb[h][ds(0, 64), ds(qc * 512, 512)],
                            rb[:],
                            OP.mult,
                        )

                def outproj(tts, heads, accum):
                    for tt in tts:
                        fp = psf.tile([P, 512], F32, tag="f", name="f")
                        for i, h in enumerate(heads):
                            nc.tensor.matmul(
                                fp[:],
                                lhsT=o_sb[h][:, ds(tt * P, P)],
                                rhs=wo_all[:, h, :],
                                start=(i == 0),
                                stop=(i == len(heads) - 1),
                            )
                        ob = opool.tile([P, 512], F32, tag="ob", name="ob")
                        nc.vector.tensor_copy(ob[:], fp[:])
                        if accum:
                            nc.gpsimd.dma_start(
                                out=t["out"][ds(tt * P, P), :], in_=ob[:],
                                accum_op=OP.add,
                            )
                        else:
                            nc.sync.dma_start(out=t["out"][ds(tt * P, P), :], in_=ob[:])

                for h in range(HL):
                    for qcp in range(2):
                        attn_unit(h, qcp)
                        if h == 3 and qcp == 0:
                            outproj(range(8), (2, 3), True)
                    if h == 1:
                        outproj(range(NTT), (0, 1), False)
                outproj(range(8, NTT), (2, 3), True)


def _build(reps=1):
    if reps in _prog_cache:
        return _prog_cache[reps]
    nc = bacc.Bacc(
        "TRN2",
        target_bir_lowering=False,
        debug=False,
        enable_asserts=False,
        num_devices=NCORES,
    )
    t = {}
    for name, shape, dt in (
        ("xqT", (D_MODEL, M), BF16),
        ("xkT", (D_MODEL, M), BF16),
        ("xvT", (D_MODEL, M), BF16),
        ("wq", (D_MODEL, 256), BF16),
        ("wk", (D_MODEL, 256), BF16),
        ("wv", (D_MODEL, VW), BF16),
        ("wo", (256, 512), BF16),
        ("bmisc", (128, 264), F32),
        ("misc", (1, 640), F32R),
    ):
        t[name] = nc.dram_tensor(name, shape, dt, kind="ExternalInput").ap()
    t["out"] = nc.dram_tensor("out", (M, D_MODEL), F32, kind="ExternalOutput").ap()

    with tile.TileContext(nc) as tc:
        for _ in range(reps):
            _emit_body(nc, tc, t)
    nc.compile()
    _prog_cache[reps] = (nc, t)
    return _prog_cache[reps]


def shard_inputs(query, key, value, Wq, bq, Wk, bk, Wv, bv, Wo, bo):
    query, key, value, Wq, bq, Wk, bk, Wv, bv, Wo, bo = (
        np.asarray(a, dtype=np.float32)
        for a in (query, key, value, Wq, bq, Wk, bk, Wv, bv, Wo, bo)
    )
    bf = ml_dtypes.bfloat16
    in_maps = []
    for b in range(B):
        xqT = np.ascontiguousarray(query[b].T).astype(bf)
        xkT = np.ascontiguousarray(key[b].T).astype(bf)
        xvT = np.ascontiguousarray(value[b].T).astype(bf)
        for g in range(GROUPS):
            hs = slice(g * 256, (g + 1) * 256)
            wv_ext = np.zeros((D_MODEL, VW), np.float32)
            onespat = np.zeros((VW,), np.float32)
            for i in range(HL):
                gh = g * HL + i
                wv_ext[:, i * 65 : i * 65 + 64] = Wv[:, gh * 64 : (gh + 1) * 64]
                onespat[i * 65 + 64] = 1.0
            bmisc = np.zeros((128, 264), np.float32)
            bmisc[:, 0:2] = bq[hs].reshape(2, 128).T
            bmisc[:, 2:4] = bk[hs].reshape(2, 128).T
            bmisc[:, 4:] = onespat
            misc = np.zeros((1, 640), np.float32)
            misc[0, 512:640] = 1.0
            in_maps.append(
                {
                    "xqT": xqT,
                    "xkT": xkT,
                    "xvT": xvT,
                    "wq": np.ascontiguousarray(Wq[:, hs]).astype(bf),
                    "wk": np.ascontiguousarray(Wk[:, hs]).astype(bf),
                    "wv": wv_ext.astype(bf),
                    "wo": np.ascontiguousarray(Wo[hs, :]).astype(bf),
                    "bmisc": bmisc,
                    "misc": misc,
                }
            )
    return in_maps


def unshard_outputs(results, c_epilogue):
    return np.stack(
        [
            results[2 * b]["out"] + results[2 * b + 1]["out"] + c_epilogue
            for b in range(B)
        ]
    )


def kernel(query, key, value, Wq, bq, Wk, bk, Wv, bv, Wo, bo):
    nc, _ = _build(reps=1)
    in_maps = shard_inputs(query, key, value, Wq, bq, Wk, bk, Wv, bv, Wo, bo)
    res = run_bass_kernel_spmd(nc, in_maps, core_ids=list(range(NCORES)))
    c = (
        np.asarray(bv, np.float32) @ np.asarray(Wo, np.float32)
        + np.asarray(bo, np.float32)
    ).astype(np.float32)
    return unshard_outputs(res.results, c)


# revision 13
# speedup vs baseline: 1383.5457x; 1.4571x over previous
"""Multi-head attention distributed over 8 Trainium2 NeuronCores.

Sharding: core = (batch b, head-group g); each core computes 4 heads of
one batch end-to-end and returns a partial [2048, 512] output; the host
sums the two group partials per batch and adds the constant epilogue
vector bv @ Wo + bo (exact, since softmax rows sum to 1).

v4: all matmul operands are bf16 (host ships bf16 X^T and weights;
separate LDWEIGHTS amortizes/hides, input DMA halves). K^T is stored
per-head zero-padded ([128, 2048] with the other head's partition half
zeroed) so every matmul runs K=128 — no PE tiling-mode switches and no
partition shifts. PSUM accumulation and the softmax-sum/reciprocal path
stay f32/f32r for accuracy. Scores are computed transposed so the exp
output P^T feeds attn@V directly; a ones-column in V yields softmax
sums for free; 1/sum is applied via a PE outer-product broadcast.

Schedule: head-pair-0 K/Q projections sweep kt-outer over all 8 PSUM
banks behind the streaming input DMAs; V and head-pair-1 projections
run k-inner on the 2 out-proj/norm banks overlapping early attention.
Attention runs one (head, q-pair) unit at a time; normalization follows
each unit; output projection is split (heads 0/1 store, heads 2/3
CCE-accumulate into DRAM) and overlaps later attention.
"""

import numpy as np
import ml_dtypes

import concourse.bacc as bacc
import concourse.mybir as mybir
import concourse.tile as tile
from concourse.bass import ds
from concourse.bass_utils import run_bass_kernel_spmd

D_MODEL, DQ, DV, H = 512, 64, 64, 8
B, M = 4, 2048
NCORES, GROUPS = 8, 2
HL = H // GROUPS            # heads per core
VW = HL * (DV + 1)          # V width incl. ones columns = 260
SCALE = float(1.0 / np.sqrt(np.float32(M)))
NKT = D_MODEL // 128        # 4 contraction tiles over d_model
NTT = M // 128              # 16 token tiles
NQC = M // 512              # 4 query chunks of 512

F32 = mybir.dt.float32
F32R = mybir.dt.float32r
BF16 = mybir.dt.bfloat16
AF = mybir.ActivationFunctionType
OP = mybir.AluOpType

_prog_cache = {}


def _emit_body(nc, tc, t):
    P = 128

    with (
        tc.tile_pool(name="consts", bufs=1) as cpool,
        tc.tile_pool(name="persist", bufs=1) as ppool,
    ):
        wq_all = cpool.tile([P, NKT, 256], BF16, tag="wq", name="wq_all")
        wk_all = cpool.tile([P, NKT, 256], BF16, tag="wk", name="wk_all")
        wv_all = cpool.tile([P, NKT, VW], BF16, tag="wv", name="wv_all")
        wo_all = cpool.tile([P, HL, 512], BF16, tag="wo", name="wo_all")
        bmisc = cpool.tile([P, 264], F32, tag="bmisc", name="bmisc")
        misc = cpool.tile([1, 640], F32R, tag="misc", name="misc")
        bqk = bmisc[:, ds(0, 4)]
        onespat = bmisc[:, ds(4, VW)]
        ones = misc[ds(0, 1), ds(512, P)]

        for kt in range(NKT):
            nc.sync.dma_start(out=wk_all[:, kt, :], in_=t["wk"][ds(kt * P, P), :])
            nc.sync.dma_start(out=wq_all[:, kt, :], in_=t["wq"][ds(kt * P, P), :])
            nc.sync.dma_start(out=wv_all[:, kt, :], in_=t["wv"][ds(kt * P, P), :])
        for h in range(HL):
            nc.sync.dma_start(out=wo_all[ds(0, 64), h, :], in_=t["wo"][ds(h * 64, 64), :])
        nc.sync.dma_start(out=bmisc[:], in_=t["bmisc"][:, :])
        nc.sync.dma_start(out=misc[:], in_=t["misc"][:, :])
        nc.vector.memset(wo_all[ds(64, 64), :, :], 0.0)

        # persistent activations (kTh per-head zero-padded; o_sb zero-padded)
        qT = [ppool.tile([P, M], BF16, tag=f"qT{i}", name=f"qT{i}") for i in range(2)]
        kTh = [ppool.tile([P, M], BF16, tag=f"kTh{i}", name=f"kTh{i}") for i in range(HL)]
        v_all = ppool.tile([P, NTT, VW], BF16, tag="v", name="v_all")
        o_sb = [ppool.tile([P, M], BF16, tag=f"o{h}", name=f"osb{h}") for h in range(HL)]
        for h in range(HL):
            z0, z1 = (64, 64) if h % 2 == 0 else (0, 64)
            nc.vector.memset(kTh[h][ds(z0, z1), :], 0.0)
            nc.vector.memset(o_sb[h][ds(64, 64), :], 0.0)

        with tc.tile_pool(name="xc", bufs=8) as xc_pool:
            xch = {}
            # ---- head-pair-0 K and Q projections: kt-outer over 8 banks ----
            with tc.tile_pool(name="psq8", bufs=1, space="PSUM") as psq8:
                pss = {
                    (w, cg): psq8.tile([P, 1024], F32, tag=f"p{w}{cg}", name=f"p{w}{cg}")
                    for w in ("k", "q")
                    for cg in range(2)
                }
                for kt in range(NKT):
                    for w, w_all, xname in (("k", wk_all, "xkT"), ("q", wq_all, "xqT")):
                        c = xc_pool.tile([P, M], BF16, tag="xc", name="xc")
                        nc.sync.dma_start(out=c[:], in_=t[xname][ds(kt * P, P), :])
                        xch[(w, kt)] = c
                        for qc in range(NQC):
                            nc.tensor.matmul(
                                pss[(w, qc // 2)][:, ds((qc % 2) * 512, 512)],
                                lhsT=w_all[:, kt, ds(0, P)],
                                rhs=c[:, ds(qc * 512, 512)],
                                start=(kt == 0),
                                stop=(kt == NKT - 1),
                            )
                for cg in range(2):
                    nc.vector.tensor_scalar(
                        qT[0][:, ds(cg * 1024, 1024)],
                        pss[("q", cg)][:], bqk[:, ds(0, 1)], None, OP.add,
                    )
                    nc.vector.tensor_scalar(
                        kTh[0][ds(0, 64), ds(cg * 1024, 1024)],
                        pss[("k", cg)][ds(0, 64), :], bqk[ds(0, 64), ds(2, 1)], None, OP.add,
                    )
                    nc.vector.tensor_scalar(
                        kTh[1][ds(64, 64), ds(cg * 1024, 1024)],
                        pss[("k", cg)][ds(64, 64), :], bqk[ds(64, 64), ds(2, 1)], None, OP.add,
                    )

            with (
                tc.tile_pool(name="pT", bufs=6) as pt_pool,
                tc.tile_pool(name="fin", bufs=1) as fpool,
                tc.tile_pool(name="outb", bufs=3) as opool,
                tc.tile_pool(name="psatt", bufs=1, space="PSUM") as psa,
                tc.tile_pool(name="psfin", bufs=1, space="PSUM") as psf,
            ):
                sr = fpool.tile([P, P], F32R, tag="sr", name="sr")
                nc.scalar.activation(sr[ds(0, 1), ds(0, 1)], misc[ds(0, 1), ds(0, 1)], AF.Exp)

                # V projection: k-inner on the psf "f" bank
                xv = []
                for kt in range(NKT):
                    c = xc_pool.tile([P, M], BF16, tag="xv", name="xv", bufs=4)
                    nc.sync.dma_start(out=c[:], in_=t["xvT"][ds(kt * P, P), :])
                    xv.append(c)
                for tt in range(NTT):
                    psv = psf.tile([P, 512], F32, tag="f", name="pv")
                    for kt in range(NKT):
                        nc.tensor.matmul(
                            psv[:, ds(0, VW)],
                            lhsT=xv[kt][:, ds(tt * P, P)],
                            rhs=wv_all[:, kt, :],
                            start=(kt == 0),
                            stop=(kt == NKT - 1),
                        )
                    nc.vector.tensor_tensor(
                        v_all[:, tt, :], psv[:, ds(0, VW)], onespat[:], OP.add
                    )

                def proj_dq1():
                    # head-pair-1 K/Q projections: k-inner on the psf "rb" bank
                    for w, w_all, bcol in (("k", wk_all, 2), ("q", wq_all, 0)):
                        for qc in range(NQC):
                            ps = psf.tile([P, 512], F32, tag="rb", name="pp")
                            for kt in range(NKT):
                                nc.tensor.matmul(
                                    ps[:],
                                    lhsT=w_all[:, kt, ds(P, P)],
                                    rhs=xch[(w, kt)][:, ds(qc * 512, 512)],
                                    start=(kt == 0),
                                    stop=(kt == NKT - 1),
                                )
                            if w == "q":
                                nc.vector.tensor_scalar(
                                    qT[1][:, ds(qc * 512, 512)],
                                    ps[:], bqk[:, ds(1, 1)], None, OP.add,
                                )
                            else:
                                nc.vector.tensor_scalar(
                                    kTh[2][ds(0, 64), ds(qc * 512, 512)],
                                    ps[ds(0, 64), :], bqk[ds(0, 64), ds(3, 1)], None, OP.add,
                                )
                                nc.vector.tensor_scalar(
                                    kTh[3][ds(64, 64), ds(qc * 512, 512)],
                                    ps[ds(64, 64), :], bqk[ds(64, 64), ds(3, 1)], None, OP.add,
                                )

                def attn_unit(h, qcp):
                    hp = h // 2
                    po = [
                        psa.tile([65, 512], F32, tag=f"po{qci}", name=f"po{qci}")
                        for qci in range(2)
                    ]
                    for j in range(NTT):
                        sps = psa.tile([P, 1024], F32, tag="ps", name="ps", bufs=2)
                        for qci in range(2):
                            qc = qcp * 2 + qci
                            nc.tensor.matmul(
                                sps[:, ds(qci * 512, 512)],
                                lhsT=kTh[h][:, ds(j * P, P)],
                                rhs=qT[hp][:, ds(qc * 512, 512)],
                                start=True,
                                stop=True,
                            )
                        pt = pt_pool.tile([P, 1024], BF16, tag="pt", name="pt")
                        nc.scalar.activation(pt[:], sps[:], AF.Exp, scale=SCALE)
                        for qci in range(2):
                            nc.tensor.matmul(
                                po[qci][:],
                                lhsT=v_all[:, j, ds(h * 65, 65)],
                                rhs=pt[:, ds(qci * 512, 512)],
                                start=(j == 0),
                                stop=(j == NTT - 1),
                            )
                    for qci in range(2):
                        qc = qcp * 2 + qci
                        nc.vector.tensor_copy(
                            o_sb[h][ds(0, 64), ds(qc * 512, 512)], po[qci][ds(0, 64), :]
                        )
                        srow = fpool.tile([1, 512], F32R, tag="srow", name="srow", bufs=2)
                        nc.vector.tensor_copy(srow[:], po[qci][ds(64, 1), :])
                        idx = h * 4 + qc
                        nc.sync.dma_start(out=sr[:, ds(idx * 4, 4)], in_=srow[:])
                        with nc.allow_low_precision(reason="f32r == f32 bits"):
                            nc.vector.reciprocal(
                                sr[:, ds(64 + idx * 4, 4)], sr[:, ds(idx * 4, 4)]
                            )
                        rr = fpool.tile([1, 512], F32R, tag="rrow", name="rrow", bufs=2)
                        nc.sync.dma_start(out=rr[:], in_=sr[:, ds(64 + idx * 4, 4)])
                        rb = psf.tile([64, 512], F32, tag="rb", name="rb")
                        nc.tensor.matmul(
                            rb[:],
                            lhsT=ones[ds(0, 1), ds(0, 64)],
                            rhs=rr[ds(0, 1), :],
                            start=True,
                            stop=True,
                        )
                        nc.vector.tensor_tensor(
                            o_sb[h][ds(0, 64), ds(qc * 512, 512)],
                            o_sb[h][ds(0, 64), ds(qc * 512, 512)],
                            rb[:],
                            OP.mult,
                        )

                def outproj(tts, heads, accum):
                    for tt in tts:
                        fp = psf.tile([P, 512], F32, tag="f", name="f")
                        for i, h in enumerate(heads):
                            nc.tensor.matmul(
                                fp[:],
                                lhsT=o_sb[h][:, ds(tt * P, P)],
                                rhs=wo_all[:, h, :],
                                start=(i == 0),
                                stop=(i == len(heads) - 1),
                            )
                        ob = opool.tile([P, 512], F32, tag="ob", name="ob")
                        nc.vector.tensor_copy(ob[:], fp[:])
                        if accum:
                            nc.gpsimd.dma_start(
                                out=t["out"][ds(tt * P, P), :], in_=ob[:],
                                accum_op=OP.add,
                            )
                        else:
                            nc.sync.dma_start(out=t["out"][ds(tt * P, P), :], in_=ob[:])

                for h in range(HL):
                    for qcp in range(2):
                        attn_unit(h, qcp)
                        if h == 3 and qcp == 0:
                            outproj(range(8), (2, 3), True)
                    if h == 0:
                        proj_dq1()
                    if h == 1:
                        outproj(range(NTT), (0, 1), False)
                outproj(range(8, NTT), (2, 3), True)


def _build(reps=1):
    if reps in _prog_cache:
        return _prog_cache[reps]
    nc = bacc.Bacc(
        "TRN2",
        target_bir_lowering=False,
        debug=False,
        enable_asserts=False,
        num_devices=NCORES,
    )
    t = {}
    for name, shape, dt in (
        ("xqT", (D_MODEL, M), BF16),
        ("xkT", (D_MODEL, M), BF16),
        ("xvT", (D_MODEL, M), BF16),
        ("wq", (D_MODEL, 256), BF16),
        ("wk", (D_MODEL, 256), BF16),
        ("wv", (D_MODEL, VW), BF16),
        ("wo", (256, 512), BF16),
        ("bmisc", (128, 264), F32),
        ("misc", (1, 640), F32R),
    ):
        t[name] = nc.dram_tensor(name, shape, dt, kind="ExternalInput").ap()
    t["out"] = nc.dram_tensor("out", (M, D_MODEL), F32, kind="ExternalOutput").ap()

    with tile.TileContext(nc) as tc:
        for _ in range(reps):
            _emit_body(nc, tc, t)
    nc.compile()
    _prog_cache[reps] = (nc, t)
    return _prog_cache[reps]


def shard_inputs(query, key, value, Wq, bq, Wk, bk, Wv, bv, Wo, bo):
    query, key, value, Wq, bq, Wk, bk, Wv, bv, Wo, bo = (
        np.asarray(a, dtype=np.float32)
        for a in (query, key, value, Wq, bq, Wk, bk, Wv, bv, Wo, bo)
    )
    bf = ml_dtypes.bfloat16
    in_maps = []
    for b in range(B):
        xqT = np.ascontiguousarray(query[b].T).astype(bf)
        xkT = np.ascontiguousarray(key[b].T).astype(bf)
        xvT = np.ascontiguousarray(value[b].T).astype(bf)
        for g in range(GROUPS):
            hs = slice(g * 256, (g + 1) * 256)
            wv_ext = np.zeros((D_MODEL, VW), np.float32)
            onespat = np.zeros((VW,), np.float32)
            for i in range(HL):
                gh = g * HL + i
                wv_ext[:, i * 65 : i * 65 + 64] = Wv[:, gh * 64 : (gh + 1) * 64]
                onespat[i * 65 + 64] = 1.0
            bmisc = np.zeros((128, 264), np.float32)
            bmisc[:, 0:2] = bq[hs].reshape(2, 128).T
            bmisc[:, 2:4] = bk[hs].reshape(2, 128).T
            bmisc[:, 4:] = onespat
            misc = np.zeros((1, 640), np.float32)
            misc[0, 512:640] = 1.0
            in_maps.append(
                {
                    "xqT": xqT,
                    "xkT": xkT,
                    "xvT": xvT,
                    "wq": np.ascontiguousarray(Wq[:, hs]).astype(bf),
                    "wk": np.ascontiguousarray(Wk[:, hs]).astype(bf),
                    "wv": wv_ext.astype(bf),
                    "wo": np.ascontiguousarray(Wo[hs, :]).astype(bf),
                    "bmisc": bmisc,
                    "misc": misc,
                }
            )
    return in_maps


def unshard_outputs(results, c_epilogue):
    return np.stack(
        [
            results[2 * b]["out"] + results[2 * b + 1]["out"] + c_epilogue
            for b in range(B)
        ]
    )


def kernel(query, key, value, Wq, bq, Wk, bk, Wv, bv, Wo, bo):
    nc, _ = _build(reps=1)
    in_maps = shard_inputs(query, key, value, Wq, bq, Wk, bk, Wv, bv, Wo, bo)
    res = run_bass_kernel_spmd(nc, in_maps, core_ids=list(range(NCORES)))
    c = (
        np.asarray(bv, np.float32) @ np.asarray(Wo, np.float32)
        + np.asarray(bo, np.float32)
    ).astype(np.float32)
    return unshard_outputs(res.results, c)
